# revision 20
# baseline (speedup 1.0000x reference)
"""Trainium2 Bass kernel for nn_DSLOModel_76570676953182.

Pipeline per batch row (2048 rows total, data-parallel over 8 cores):
  1. 1-D KNN (K=8) over 360 points, queried at every 4th point -> (90,) agg
  2. MLP 90->256->128->128(tanh)
  3. 2-layer LSTM (seq len 1, h0=c0=0, forget gate unused)
  4. Peephole quirk using c1 of the LAST batch row (row 2047 replicated to
     every core so no collective is needed)
  5. Gated refinement -> (3,)

KNN strategy (per 128-row tile, rows on partitions):
  key[j] = 2*v*x[j] - x[j]^2 = v^2 - (x[j]-v)^2 orders candidates identically
  to -|x[j]-v| (the v^2 offset is constant within a (row, query) pair), so:
    key  = stt(X, 2v_col, xsq, mult, subtract)   one op (DVE or Pool)
    top8 = max8(key)                             one op (DVE only)
    agg  = stt(key, top8[:,7], X, is_ge, mult, accum_out)  one op (Pool/DVE)
  xsq = X^2 and 2v are built once per tile on the scalar engine. Work is
  split DVE/Pool to balance engine busy time (Pool idled in the baseline).
"""

import sys

sys.path.insert(0, "/opt/trn_rl_repo")

import numpy as np

import concourse.bass as bass
import concourse.mybir as mybir
import concourse.tile as tile
from concourse.bass_utils import run_bass_kernel_spmd
from concourse.masks import make_identity

F32 = mybir.dt.float32
AF = mybir.ActivationFunctionType
ALU = mybir.AluOpType

B, N, K, DS, H, FD = 2048, 360, 8, 4, 192, 128
NQ = N // DS          # 90 queries per row
NCORES = 8
RPC = B // NCORES     # 256 rows per core
RT = RPC // 128       # 2 row-tiles per core
NR = RPC + 1          # 257 columns in phase B (rows + replicated last row)
BIG = 1e30
NSMALL = 39

# (gate_offset, chunk_offset, chunk_size) for the used LSTM gates i, g, o
GATES = [0, 2 * H, 3 * H]           # i, g, o offsets in the 4H gate vector
CHUNKS = [(0, 128), (128, 64)]      # H=192 split into partition chunks


def build_nc():
    nc = bass.Bass(target_bir_lowering=False, trn_type="TRN2")

    # ---- dram I/O ----
    xs = nc.dram_tensor("xs", [RPC, N], F32, kind="ExternalInput")
    xlast = nc.dram_tensor("xlast", [1, N], F32, kind="ExternalInput")
    w1aT = nc.dram_tensor("w1aT", [NQ, 256], F32, kind="ExternalInput")
    w1bT = nc.dram_tensor("w1bT", [256, 128], F32, kind="ExternalInput")
    w2T = nc.dram_tensor("w2T", [128, FD], F32, kind="ExternalInput")
    wih0T = nc.dram_tensor("wih0T", [FD, 4 * H], F32, kind="ExternalInput")
    wih1T = nc.dram_tensor("wih1T", [H, 4 * H], F32, kind="ExternalInput")
    wzrpT = nc.dram_tensor("wzrpT", [128, 2 * H + 2 * H + 6], F32, kind="ExternalInput")
    small = nc.dram_tensor("small", [128, NSMALL], F32, kind="ExternalInput")
    out = nc.dram_tensor("out", [RPC, 3], F32, kind="ExternalOutput")

    with tile.TileContext(nc) as tc:
        with (
            tc.tile_pool(name="wpool", bufs=1) as wp_,
            tc.tile_pool(name="apool", bufs=2) as ap_,
            tc.tile_pool(name="qpool", bufs=8) as qp_,
            tc.tile_pool(name="bpool", bufs=2) as bp_,
            tc.tile_pool(name="psum", bufs=6, space="PSUM") as ps_,
            tc.tile_pool(name="psumT", bufs=2, space="PSUM") as psT_,
        ):
            # ---------- input DMAs first: nothing can start until X lands ----------
            Xs = []
            for t in range(RT):
                X = ap_.tile([128, N], F32, tag="X", name=f"X{t}")
                nc.sync.dma_start(X[:], xs[t * 128 : (t + 1) * 128, :])
                Xs.append(X)
            xlast_sb = ap_.tile([1, N], F32, tag="xlast")
            nc.sync.dma_start(xlast_sb[:], xlast[:])
            vcol = ap_.tile([NQ, 1], F32, tag="vcol")
            nc.sync.dma_start(vcol[:], xlast.rearrange("one (q s) -> q (one s)", s=DS)[:, 0:1])

            # ---------- persistent weights / constants ----------
            ident = wp_.tile([128, 128], F32, tag="ident")
            make_identity(nc, ident[:])

            w1aT_sb = wp_.tile([NQ, 256], F32, tag="w1aT")
            nc.sync.dma_start(w1aT_sb[:], w1aT[:])
            w1bT_sb = [wp_.tile([128, 128], F32, tag=f"w1bT{i}", name=f"w1bT{i}") for i in range(2)]
            nc.sync.dma_start(w1bT_sb[0][:], w1bT[0:128, :])
            nc.sync.dma_start(w1bT_sb[1][:], w1bT[128:256, :])
            w2T_sb = wp_.tile([128, FD], F32, tag="w2T")
            nc.sync.dma_start(w2T_sb[:], w2T[:])
            wih0T_sb = wp_.tile([FD, 4 * H], F32, tag="wih0T")
            nc.sync.dma_start(wih0T_sb[:], wih0T[:])
            wih1T_A = wp_.tile([128, 4 * H], F32, tag="wih1TA")
            wih1T_B = wp_.tile([64, 4 * H], F32, tag="wih1TB")
            nc.sync.dma_start(wih1T_A[:], wih1T[0:128, :])
            nc.sync.dma_start(wih1T_B[:], wih1T[128:H, :])

            wzrp_sb = wp_.tile([128, 4 * H + 6], F32, tag="wzrp")
            nc.sync.dma_start(wzrp_sb[:], wzrpT[:])
            wzT_A = wzrp_sb[:, 0:H]
            wzT_B = wzrp_sb[0:64, H : 2 * H]
            wrT_A = wzrp_sb[:, 2 * H : 3 * H]
            wrT_B = wzrp_sb[0:64, 3 * H : 4 * H]
            wpT_A = wzrp_sb[:, 4 * H : 4 * H + 3]
            wpT_B = wzrp_sb[0:64, 4 * H + 3 : 4 * H + 6]

            small_sb = wp_.tile([128, NSMALL], F32, tag="small")
            nc.sync.dma_start(small_sb[:], small[:])
            bsum0_t = wp_.tile([128, 6], F32, tag="bsum0")
            nc.vector.tensor_add(bsum0_t[:], small_sb[:, 0:6], small_sb[:, 6:12])
            bsum1_t = wp_.tile([128, 6], F32, tag="bsum1")
            nc.vector.tensor_add(bsum1_t[:], small_sb[:, 12:18], small_sb[:, 18:24])

            def bias_dict(bsum):
                res, j = {}, 0
                for goff in GATES:
                    for coff, sz in CHUNKS:
                        res[(goff, coff)] = bsum[0:sz, j : j + 1]
                        j += 1
                return res

            bias0 = bias_dict(bsum0_t)
            bias1 = bias_dict(bsum1_t)

            def chunk_cols(base):
                return [small_sb[0:sz, base + ci : base + ci + 1] for ci, (coff, sz) in enumerate(CHUNKS)]

            pwf_sb = chunk_cols(24)
            pwi_sb = chunk_cols(26)
            pwo_sb = chunk_cols(28)
            bz_sb = chunk_cols(30)
            br_sb = chunk_cols(32)
            b1a_sb = small_sb[:, 34:36]
            b1b_sb = small_sb[:, 36:37]
            b2_sb = small_sb[:, 37:38]
            bp_sb = small_sb[0:3, 38:39]

            # ---------- phase A: KNN ----------
            aggT = wp_.tile([NQ, NR], F32, tag="aggT")  # (90 x 257), /8 applied

            def knn_tile(x_ap, negv, agg, p, nq_rows, feeder=None):
                """x_ap: (p, N) sbuf; v2 = 2*query vals (p, NQ); xsq = x^2 (p, N);
                negv = -query vals (p, NQ); agg: (p, nq) out (sum of selected 8).

                Per query the key tensor only has to order candidates by
                -(x - v)^2; max8 and the threshold-select run on DVE (only
                engine with max8/stt), so key production is farmed out:
                  path a (DVE):  key = stt(x, 2v, xsq, mult, subtract)
                  path b (ACT):  sq = Square(x - v); key = Copy(-sq)
                  path c (ACT+Pool): sq = Square(x - v); key = ts_mul(sq, -1)
                """
                for qi in range(nq_rows):
                    key = qp_.tile([p, N], F32, tag="key", name="key")
                    top8 = qp_.tile([p, 8], F32, tag="top8", name="top8")
                    sq = qp_.tile([p, N], F32, tag="sq", name="sq")
                    nc.scalar.activation(
                        sq[:], x_ap, AF.Square, bias=negv[:, qi : qi + 1], scale=1.0
                    )
                    if qi % 10 in (0, 4, 7):
                        # 1-src imm tensor_scalar negate: DVE 2x perf mode
                        nc.vector.tensor_scalar_mul(key[:], sq[:], -1.0)
                    else:
                        nc.scalar.activation(key[:], sq[:], AF.Copy, bias=0.0, scale=-1.0)
                    nc.vector.max(out=top8[:], in_=key[:])
                    # select: (key >= 8th-largest) * x, accumulated per row
                    nc.vector.scalar_tensor_tensor(
                        out=sq[:], in0=key[:], scalar=top8[:, 7:8], in1=x_ap,
                        op0=ALU.is_ge, op1=ALU.mult,
                        accum_out=agg[:, qi : qi + 1],
                    )
                    if feeder is not None and qi % 2 == 1:
                        feeder()

            # replicated last row first: fills the DMA lead-in bubble and gets
            # the c_last/peephole chain off the end-of-kernel critical path
            ones = ap_.tile([1, NQ], F32, tag="ones")
            nc.vector.memset(ones[:], 1.0)
            xq_ps = psT_.tile([NQ, N], F32, tag="pst", name="xqps")
            nc.tensor.matmul(xq_ps[:], ones[:], xlast_sb[:])
            xq = ap_.tile([NQ, N], F32, tag="xq")
            nc.scalar.activation(xq[:], xq_ps[:], AF.Copy, bias=0.0, scale=1.0)
            v2l = ap_.tile([NQ, 1], F32, tag="v2l")
            nc.vector.tensor_scalar_mul(v2l[:], vcol[:], 2.0)

            xsql = ap_.tile([NQ, N], F32, tag="xsql")
            nc.scalar.activation(xsql[:], xq[:], AF.Square, bias=0.0, scale=1.0)
            keyl = ap_.tile([NQ, N], F32, tag="keyl")
            top8l = ap_.tile([NQ, 8], F32, tag="top8l")
            agg_last = ap_.tile([NQ, 1], F32, tag="agg_last")
            nc.vector.scalar_tensor_tensor(
                out=keyl[:], in0=xq[:], scalar=v2l[:, 0:1], in1=xsql[:],
                op0=ALU.mult, op1=ALU.subtract,
            )
            nc.vector.max(out=top8l[:], in_=keyl[:])
            nc.vector.scalar_tensor_tensor(
                out=keyl[:], in0=keyl[:], scalar=top8l[:, 7:8], in1=xq[:],
                op0=ALU.is_ge, op1=ALU.mult,
                accum_out=agg_last[:, 0:1],
            )
            nc.vector.tensor_scalar_mul(aggT[:, RPC : RPC + 1], agg_last[:], 0.125)

            # ---------- phase B as interleavable generators ----------
            # Emitted in column slices and fed one step per KNN query so the
            # per-engine instruction streams never stall on phase B's long
            # dependency chain: the c_last 1-column chain drains under tile0's
            # KNN, columns 0:128 under tile1's; only 128:256 remain as tail.
            peep = {}
            out_sb = wp_.tile([3, RPC], F32, tag="out_sb")

            def lstm_g(rhs_chunks, wT_chunks, biases, lname, cw, tg):
                acts = {}
                for goff in GATES:
                    for coff, sz in CHUNKS:
                        ps = ps_.tile([sz, cw], F32, tag="ps", name=f"psg_{lname}_{goff}_{coff}{tg}")
                        for ci2, (rt_, wt_) in enumerate(zip(rhs_chunks, wT_chunks)):
                            s = goff + coff
                            nc.tensor.matmul(
                                ps[:], wt_[:, s : s + sz], rt_[:],
                                start=(ci2 == 0), stop=(ci2 == len(rhs_chunks) - 1),
                            )
                        func = AF.Tanh if goff == 2 * H else AF.Sigmoid
                        a = wp_.tile([sz, cw], F32, tag=f"act_{lname}_{goff}_{coff}{tg}")
                        nc.scalar.activation(a[:], ps[:], func, bias=biases[(goff, coff)], scale=1.0)
                        acts[(goff, coff)] = a
                        yield 1
                hs, cs = [], []
                for coff, sz in CHUNKS:
                    c = wp_.tile([sz, cw], F32, tag=f"c_{lname}_{coff}{tg}")
                    nc.vector.tensor_mul(c[:], acts[(0, coff)][:], acts[(2 * H, coff)][:])
                    tc_ = wp_.tile([sz, cw], F32, tag=f"tc_{lname}_{coff}{tg}")
                    nc.scalar.activation(tc_[:], c[:], AF.Tanh, bias=0.0, scale=1.0)
                    h = wp_.tile([sz, cw], F32, tag=f"h_{lname}_{coff}{tg}")
                    nc.vector.tensor_mul(h[:], acts[(3 * H, coff)][:], tc_[:])
                    hs.append(h)
                    cs.append(c)
                    yield 1
                return hs, cs

            def phase_b_gen(c0, cw, tg, full):
                """MLP+LSTM+refinement on aggT[:, c0:c0+cw]. full=False: stop
                after LSTM-1 cell state, publish peephole scalar columns into
                `peep`. full=True: consume `peep`, write out_sb[:, c0:c0+cw]."""
                asl = aggT[:, c0 : c0 + cw]
                x1 = []
                for m in range(2):
                    ps = ps_.tile([128, cw], F32, tag="ps", name=f"psL1{tg}{m}")
                    nc.tensor.matmul(ps[:], w1aT_sb[:, m * 128 : (m + 1) * 128], asl)
                    t_ = wp_.tile([128, cw], F32, tag=f"x1_{m}{tg}")
                    nc.scalar.activation(t_[:], ps[:], AF.Relu, bias=b1a_sb[:, m : m + 1], scale=1.0)
                    x1.append(t_)
                    yield 1
                ps2 = ps_.tile([128, cw], F32, tag="ps", name=f"psL2{tg}")
                nc.tensor.matmul(ps2[:], w1bT_sb[0][:], x1[0][:], start=True, stop=False)
                nc.tensor.matmul(ps2[:], w1bT_sb[1][:], x1[1][:], start=False, stop=True)
                x2 = wp_.tile([128, cw], F32, tag=f"x2{tg}")
                nc.scalar.activation(x2[:], ps2[:], AF.Relu, bias=b1b_sb[:, 0:1], scale=1.0)
                yield 1
                ps3 = ps_.tile([128, cw], F32, tag="ps", name=f"psL3{tg}")
                nc.tensor.matmul(ps3[:], w2T_sb[:], x2[:])
                ftmp = wp_.tile([128, cw], F32, tag=f"ftmp{tg}")
                nc.scalar.activation(ftmp[:], ps3[:], AF.Relu, bias=b2_sb[:, 0:1], scale=1.0)
                feat = wp_.tile([128, cw], F32, tag=f"feat{tg}")
                nc.scalar.activation(feat[:], ftmp[:], AF.Tanh, bias=0.0, scale=1.0)
                yield 1
                h0c, _ = yield from lstm_g([feat], [wih0T_sb], bias0, "l0", cw, tg)
                h1c, c1c = yield from lstm_g(h0c, [wih1T_A, wih1T_B], bias1, "l1", cw, tg)
                if not full:
                    for ci, (coff, sz) in enumerate(CHUNKS):
                        ccol = c1c[ci][:, 0:1]
                        tcl = wp_.tile([sz, 1], F32, tag=f"tcl_{coff}")
                        nc.scalar.activation(tcl[:], ccol, AF.Tanh, bias=0.0, scale=1.0)
                        peep[("tcl", ci)] = tcl
                        peep[("c", ci)] = ccol
                        for nm, pw in (("f", pwf_sb), ("i", pwi_sb), ("o", pwo_sb)):
                            pcol = wp_.tile([sz, 1], F32, tag=f"p_{nm}_{coff}")
                            nc.vector.tensor_mul(pcol[:], pw[ci], ccol)
                            peep[(nm, ci)] = pcol
                        yield 1
                    return
                temp = []
                for ci, (coff, sz) in enumerate(CHUNKS):
                    ccol = peep[("c", ci)]
                    gates = {}
                    for nm in ("f", "i", "o"):
                        g = wp_.tile([sz, cw], F32, tag=f"pg_{nm}_{coff}{tg}")
                        nc.scalar.activation(
                            g[:], h1c[ci][:], AF.Sigmoid, bias=peep[(nm, ci)][:, 0:1], scale=1.0
                        )
                        gates[nm] = g
                        yield 1
                    u = wp_.tile([sz, cw], F32, tag=f"u_{coff}{tg}")
                    nc.vector.tensor_scalar_mul(u[:], gates["f"][:], ccol)
                    cell = wp_.tile([sz, cw], F32, tag=f"cell_{coff}{tg}")
                    nc.vector.scalar_tensor_tensor(
                        out=cell[:], in0=gates["i"][:], scalar=peep[("tcl", ci)][:, 0:1],
                        in1=u[:], op0=ALU.mult, op1=ALU.add,
                    )
                    tcell = wp_.tile([sz, cw], F32, tag=f"tcell_{coff}{tg}")
                    nc.scalar.activation(tcell[:], cell[:], AF.Tanh, bias=0.0, scale=1.0)
                    tmp_ = wp_.tile([sz, cw], F32, tag=f"temp_{coff}{tg}")
                    nc.vector.tensor_mul(tmp_[:], gates["o"][:], tcell[:])
                    temp.append(tmp_)
                    yield 1
                zr = []
                for wA, wB, bs, ln in ((wzT_A, wzT_B, bz_sb, "z"), (wrT_A, wrT_B, br_sb, "r")):
                    outs = []
                    for ci, (coff, sz) in enumerate(CHUNKS):
                        ps = ps_.tile([sz, cw], F32, tag="ps", name=f"pszr_{ln}_{coff}{tg}")
                        nc.tensor.matmul(ps[:], wA[:, coff : coff + sz], temp[0][:], start=True, stop=False)
                        nc.tensor.matmul(ps[:], wB[:, coff : coff + sz], temp[1][:], start=False, stop=True)
                        g = wp_.tile([sz, cw], F32, tag=f"zr_{ln}_{coff}{tg}")
                        nc.scalar.activation(g[:], ps[:], AF.Sigmoid, bias=bs[ci], scale=1.0)
                        outs.append(g)
                        yield 1
                    zr.append(outs)
                zg, rg = zr
                y = []
                for ci, (coff, sz) in enumerate(CHUNKS):
                    y_ = wp_.tile([sz, cw], F32, tag=f"y_{coff}{tg}")
                    nc.vector.tensor_mul(y_[:], rg[ci][:], temp[ci][:])
                    nc.vector.tensor_mul(y_[:], y_[:], zg[ci][:])
                    y.append(y_)
                    yield 1
                ps_out = ps_.tile([3, cw], F32, tag="ps", name=f"psout{tg}")
                nc.tensor.matmul(ps_out[:], wpT_A[:], y[0][:], start=True, stop=False)
                nc.tensor.matmul(ps_out[:], wpT_B[:], y[1][:], start=False, stop=True)
                nc.scalar.activation(out_sb[:, c0 : c0 + cw], ps_out[:], AF.Identity, bias=bp_sb, scale=1.0)

            genq = []

            def feeder():
                while genq:
                    if next(genq[0], None) is not None:
                        return
                    genq.pop(0)

            genq.append(phase_b_gen(RPC, 1, "L", False))
            for t in range(RT):
                X = Xs[t]
                negv = ap_.tile([128, NQ], F32, tag="negv")
                nc.scalar.activation(negv[:], X[:, 0:N:DS], AF.Copy, bias=0.0, scale=-1.0)
                agg = ap_.tile([128, NQ], F32, tag="agg")
                knn_tile(X[:], negv, agg, 128, NQ, feeder)
                # transpose (128 x 90) -> (90 x 128) into aggT columns, scale 1/8
                tp = psT_.tile([NQ, 128], F32, tag="pst", name="tp")
                nc.tensor.transpose(tp[:], agg[:, 0:NQ], ident[:, 0:128])
                nc.scalar.activation(
                    aggT[:, t * 128 : (t + 1) * 128], tp[:], AF.Copy, bias=0.0, scale=0.125
                )
                if t == 0:
                    genq.append(phase_b_gen(0, 128, "A", True))
            while genq:
                if next(genq[0], None) is None:
                    genq.pop(0)
            for _ in phase_b_gen(128, RPC - 128, "B", True):
                pass

            nc.sync.dma_start(out.rearrange("r c -> c r"), out_sb[:])

    _split_excess_waits(nc)
    return nc


def _split_excess_waits(nc, max_waits=1):
    """walrus's inline sync encoding allows only 2 waits on compute
    instructions; hoist overflow waits onto same-engine drain clones."""
    import copy

    import concourse.mybir as mybir

    proto = None
    for bb in nc.main_func.blocks:
        for ins in bb.instructions:
            if type(ins).__name__ == "InstDrain":
                proto = ins
                break
        if proto:
            break
    assert proto is not None
    n = 0
    for bb in nc.main_func.blocks:
        lst = bb.instructions
        i = 0
        while i < len(lst):
            ins = lst[i]
            si = ins.sync_info
            waits = list(si.on_wait) if si and si.on_wait else []
            if len(waits) > max_waits:
                keep = waits[-max_waits:]
                over = waits[:-max_waits]
                ins.sync_info = mybir.SyncInfo(
                    on_wait=keep, on_update=list(si.on_update or [])
                )
                carriers = []
                while over:
                    chunk, over = over[:max_waits], over[max_waits:]
                    c = copy.deepcopy(proto)
                    n += 1
                    c.name = f"I-waitfix-{n}"
                    c.engine = ins.engine
                    c.sync_info = mybir.SyncInfo(on_wait=chunk, on_update=[])
                    carriers.append(c)
                lst[i:i] = carriers
                i += len(carriers)
            i += 1


_NC_CACHE = {}


def _get_nc():
    if "nc" not in _NC_CACHE:
        _NC_CACHE["nc"] = build_nc()
    return _NC_CACHE["nc"]


def _prep_in_maps(inputs):
    f32c = lambda a: np.ascontiguousarray(np.asarray(a), dtype=np.float32)
    X = f32c(inputs["lidar_batch"])

    sm = np.zeros((128, NSMALL), np.float32)

    def put_gate_chunks(vec, base):  # (4H,) -> 6 chunk columns
        j = 0
        for goff in GATES:
            for coff, sz in CHUNKS:
                s = goff + coff
                sm[0:sz, base + j] = vec[s : s + sz]
                j += 1

    def put_chunks(vec, base):  # (H,) -> 2 chunk columns
        for ci, (coff, sz) in enumerate(CHUNKS):
            sm[0:sz, base + ci] = vec[coff : coff + sz]

    put_gate_chunks(f32c(inputs["bih0"]), 0)
    put_gate_chunks(f32c(inputs["bhh0"]), 6)
    put_gate_chunks(f32c(inputs["bih1"]), 12)
    put_gate_chunks(f32c(inputs["bhh1"]), 18)
    put_chunks(f32c(inputs["pwf"]), 24)
    put_chunks(f32c(inputs["pwi"]), 26)
    put_chunks(f32c(inputs["pwo"]), 28)
    put_chunks(f32c(inputs["bz"]), 30)
    put_chunks(f32c(inputs["br"]), 32)
    b1a = f32c(inputs["b1a"])
    sm[:, 34] = b1a[0:128]
    sm[:, 35] = b1a[128:256]
    sm[:, 36] = f32c(inputs["b1b"])
    sm[:, 37] = f32c(inputs["b2"])
    sm[0:3, 38] = f32c(inputs["bp"])

    wzT = f32c(np.asarray(inputs["wz"]).T)
    wrT = f32c(np.asarray(inputs["wr"]).T)
    wpT = f32c(np.asarray(inputs["wp"]).T)
    wzrp = np.zeros((128, 4 * H + 6), np.float32)
    wzrp[:, 0:H] = wzT[0:128]
    wzrp[0:64, H : 2 * H] = wzT[128:H]
    wzrp[:, 2 * H : 3 * H] = wrT[0:128]
    wzrp[0:64, 3 * H : 4 * H] = wrT[128:H]
    wzrp[:, 4 * H : 4 * H + 3] = wpT[0:128]
    wzrp[0:64, 4 * H + 3 : 4 * H + 6] = wpT[128:H]

    shared = dict(
        xlast=X[B - 1 : B].copy(),
        w1aT=f32c(np.asarray(inputs["w1a"]).T),
        w1bT=f32c(np.asarray(inputs["w1b"]).T),
        w2T=f32c(np.asarray(inputs["w2"]).T),
        wih0T=f32c(np.asarray(inputs["wih0"]).T),
        wih1T=f32c(np.asarray(inputs["wih1"]).T),
        wzrpT=wzrp,
        small=sm,
    )
    return [
        dict(shared, xs=X[c * RPC : (c + 1) * RPC].copy()) for c in range(NCORES)
    ]


def run(inputs, trace=False, **kw):
    nc = _get_nc()
    in_maps = _prep_in_maps(inputs)
    res = run_bass_kernel_spmd(nc, in_maps, list(range(NCORES)), trace=trace, **kw)
    out = np.concatenate([r["out"] for r in res.results], axis=0)
    return out, res


def kernel(**inputs):
    out, _ = run(inputs)
    return out.astype(np.float32)

